# revision 78
# baseline (speedup 1.0000x reference)
"""MultiHeadSelfAttention Trainium2 kernel (8-core SPMD), v4.

v4 over v2 (the staged baseline):
  - Attention runs as 8 PAIRS of same-A2A-half 256-col units (j and
    j+2, one strided N=512 rhs): score and AV matmuls at N=512 halve
    the PE instruction count (1183 -> ~600 matmuls) while each A2A
    half still completes after its own 4 pairs, preserving the early
    first collective. po h0/h1 live in separate PSUM banks so both AV
    accumulation groups run inline (no h1 burst); AV trails the exp
    stream by 2 kt so the in-order PE queue never stalls on a fresh
    exp. Projection chains allocate from the same PSUM ring as the
    score tiles (no dedicated pj bank, no chain serialization); QK
    chains are one 512-wide DR group, V chains use DoubleRow.
  - Loads: x as 8 half-blocks alternating sync/scalar queues (b0
    first at full HBM bandwidth); wo/xres chunked on the gpsimd queue
    (idle at startup; a big mid-kernel load once stalled rc4 folds
    ~15us behind one multi-MB descriptor).
  - Staging: merged per-unit og DMA, all staging + rc4 folds on
    gpsimd SW-DGE (HWDGE concurrent with a collective transport
    stalls ~10us), phased og-before-folds to minimize cross-engine
    ping-pong.
  - Receive: per-peer-pair og tiles (tile-granular deps let each
    normalize start on its own chunk), S rows first, recv0 pinned
    after all stagings via a wait hint (the scheduler otherwise
    reorders it in front, blocking stagings behind the collective
    semaphore), recv1 issues split across sync+scalar.
  - Tail: LN mean from stt accum_out, variance via scalar Square with
    per-partition -mu bias + accum_out (Square is a filler in every
    ACT table set), Sqrt table preloaded right after the last exp,
    per-half LN finish with stores spread over sync/scalar.
Sharding: 16 heads across 8 cores (2 heads each, "h" in {0,1}); every core
computes BOTH batches for its 2 heads. Output rows: core c owns batch c//4,
q-rows (c%4)*512..+512.

Per-core program (fp8 matmuls, f32 accum): projections from xT fp8
[D, 2b, L] with fp8 weights scaled x64 (Q,K DoubleRow chains -> qt/kt
bf16 x1/64; V DR chains -> v8 fp8 [128 kpos, 2b, 16 lt, 2h x 65+pad],
col 64 of each head block = 1.0 softmax-denominator rider, v scaled
1/256). Attention per pair: 16 kt iterations of 2 score MMs (bf16,
K=64, h-row-tiled concurrent, N=512) -> sc PSUM [128,1024]; exp to
fp8: 56/128 tiles on a custom DVE polynomial (p(s)^4 ~ lam*e^{s/8};
lam mixing across engines stays within tolerance), the rest ScalarE
ACT Exp; AV DoubleRow accumulates po0/po1 [65, 512]. Per unit: drain
po -> po_s f32, fold denominators -> [128,4], reciprocal x256, stage
raw o + 256/den rows (gpsimd f32->bf16 cast DMAs) into cc_in. Two
8-rank AllToAlls ship [8, 130, 256] bf16 shards (fp8 collectives
measured erratic). Receiver: og_mult normalizes per peer pair
(o * 256/den, fp8), out-projection DR with wo fp8 (x64), residual
out = psum/4096 + xres, LayerNorm, store.
"""

import sys

sys.path.insert(0, "/opt/trn_rl_repo")

import numpy as np
import ml_dtypes

import concourse.bass as bass
import concourse.bacc as bacc
import concourse.tile as tile
from concourse import mybir
from concourse import bass_utils
import bass_rust

BF16 = mybir.dt.bfloat16
F32 = mybir.dt.float32
FP8 = mybir.dt.float8e4
AF = mybir.ActivationFunctionType
DR = mybir.MatmulPerfMode.DoubleRow
MUL = mybir.AluOpType.mult
E4 = ml_dtypes.float8_e4m3

# custom DVE exp: p(s) = ((A3 s + A2) s + A1) s + 1;  p^4 ~ LAM * e^{s/8}
A1 = 0.031379391305728067
A2 = 0.00050919663280734283
A3 = 4.9613446254565463e-06

_PATCHED = False


def _patch_tile_drain():
    """The installed walrus rejects >1 sem wait on a Drain instruction; split
    the TileContext tail-drain waits across multiple drains."""
    global _PATCHED
    if _PATCHED:
        return
    _PATCHED = True

    def _patched(self, tick_clock, wait_clock):
        from concourse.vector_clock import ScopedClock

        probe = self.nc.sync.drain()
        wait_clock.add_sem_waits(
            probe.ins, ScopedClock({None: tick_clock.global_clock})
        )
        si = probe.ins.sync_info
        waits = list(si.on_wait or []) if si is not None else []
        if len(waits) > 1:
            si.on_wait = [waits[0]]
            for w in waits[1:]:
                d2 = self.nc.sync.drain()
                si2 = d2.ins.sync_info
                if si2 is None:
                    d2.ins.sync_info = bass_rust.SyncInfo(on_wait=[w], on_update=[])
                else:
                    si2.on_wait = [w]
        self.nc.all_engine_barrier()
        assert self.sems is not None
        popped = self.nc._tile_sem_poison_stack.pop()
        assert popped is self._sem_poison
        self.nc.clear_and_free_semaphores(list(self.sems.allocated().values()))
        self.nc.all_engine_barrier()

    tile.TileContext._drain_and_barrier = _patched


def _register_exp_op():
    """Register the polynomial-exp custom DVE op (append-only; idempotent)."""
    from concourse import dve_ops
    from concourse.dve_spec import Spec, Src0, C0, C1, C2, One, sq, lower
    from concourse.dve_uop import DveOpSpec
    from concourse.dve_ops import DveOp

    name = "EXP_S8_ANT"
    if name in dve_ops._SUB_OPCODE_FOR_NAME:
        return next(o for o in dve_ops.OPS if o.name == name)

    x = Src0
    body = sq(sq(((C2 * x + C1) * x + C0) * x + One))

    def ref(in0, in1, s0, s1, imm2):
        return ((((imm2 * in0 + s1) * in0 + s0) * in0 + 1.0) ** 2) ** 2

    spec = Spec(body=body, reference=ref)
    row = dve_ops._CUSTOM_DVE_ROW_BASE + len(dve_ops.OPS)
    shas = {}
    for ver in ("v3", "v4"):
        compiled = DveOpSpec(
            name=name, opcode=row, uops=lower(spec, ver=ver), rd1_en=False
        )
        shas[ver] = compiled.sha(ver)
    op = DveOp(name, spec, subdim=False, uops_sha=shas)
    dve_ops.OPS.append(op)
    dve_ops._SUB_OPCODE_FOR_NAME[name] = row
    dve_ops.CUSTOM_DVE_SPECS[name] = spec
    return op


def build_nc(L=2048, D=1024, eps=1e-6, trivial_gamma=False, trivial_beta=False):
    _patch_tile_drain()
    EXP_OP = _register_exp_op()

    KD = D // 128     # 8 contraction tiles over D
    LT = L // 128     # 16 kpos tiles
    NJ = L // 256     # 8 q-chunks of 256 per batch
    QW = 256          # unit q width
    QS = 512          # per-core output rows

    nc = bacc.Bacc(num_devices=8, debug=False)

    xT_d = nc.dram_tensor("xT", [D, 2 * L], FP8, kind="ExternalInput")
    wq_d = nc.dram_tensor("wq", [D, 128], FP8, kind="ExternalInput")
    wk_d = nc.dram_tensor("wk", [D, 128], FP8, kind="ExternalInput")
    wv_d = nc.dram_tensor("wv", [D, 128], FP8, kind="ExternalInput")
    wo_d = nc.dram_tensor("wo", [D, D], FP8, kind="ExternalInput")
    xres_d = nc.dram_tensor("xres", [QS, D], F32, kind="ExternalInput")
    gamma_d = nc.dram_tensor("gamma", [1, D], F32, kind="ExternalInput")
    beta_d = nc.dram_tensor("beta", [1, D], F32, kind="ExternalInput")
    out_d = nc.dram_tensor("out", [QS, D], F32, kind="ExternalOutput")

    with tile.TileContext(nc) as tc:
        with (
            tc.tile_pool(name="singles", bufs=1) as singles,
            tc.tile_pool(name="exp", bufs=10) as expp,
            tc.tile_pool(name="small", bufs=2) as small,
            tc.tile_pool(name="psum", bufs=1, space="PSUM") as psum,
            tc.tile_pool(name="dram", bufs=1, space="DRAM") as dram,
        ):
            # ---------------- loads ----------------
            xT_sb = singles.tile([128, KD, 2, L], FP8)
            wq_sb = singles.tile([128, KD, 128], FP8)
            wk_sb = singles.tile([128, KD, 128], FP8)
            wv_sb = singles.tile([128, KD, 128], FP8)
            for eng, w_sb, w_d in (
                (nc.sync, wk_sb, wk_d),
                (nc.scalar, wq_sb, wq_d),
            ):
                eng.dma_start(
                    out=w_sb, in_=w_d.ap().rearrange("(t p) m -> p t m", p=128)
                )
            # x loads: each (b, lh) 1MB block split across BOTH queues so the
            # block in flight gets the full HBM bandwidth, strictly ordered
            # b0-lh0 first (what the first chains need). wv slots in after
            # the b0-lh0 half: the V chain runs third, after K and Q.
            xT_r = xT_d.ap().rearrange("(t p) m -> p t m", p=128)
            for b in range(2):
                for lh in range(2):
                    for tq, eng in ((0, nc.sync), (4, nc.scalar)):
                        nc_sl = slice(b * L + lh * 1024, b * L + (lh + 1) * 1024)
                        eng.dma_start(
                            out=xT_sb[:, tq : tq + 4, b,
                                      lh * 1024 : (lh + 1) * 1024],
                            in_=xT_r[:, tq : tq + 4, nc_sl],
                        )
                    if b == 0 and lh == 0:
                        nc.scalar.dma_start(
                            out=wv_sb,
                            in_=wv_d.ap().rearrange("(t p) m -> p t m", p=128),
                        )
            wo_sb = singles.tile([128, KD, D], FP8)
            xres_sb = singles.tile([128, 4, D], F32)
            gb_sb = singles.tile([128, D], F32)
            bb_sb = singles.tile([128, D], F32)
            # wo/xres loads go on the GPSIMD queue: it is idle until unit-0
            # staging (~30us), is not in the startup critical path (sync/
            # scalar queues gate the first chains), and owns no per-unit
            # small DMAs that could get stuck behind a multi-MB descriptor.
            # Chunked so its own staging DMAs can slip in at boundaries.
            # Probe DMAs first: each writes a corner of wo_sb/xres_sb from
            # xT(b0, lh0), so via WAW deps the 3MB wo/xres stream stays off
            # the HBM until the first chains' data has landed (otherwise
            # the PE stalls ~13us at startup). The probed bytes are
            # overwritten by the real loads right after.
            nc.gpsimd.dma_start(
                out=wo_sb[0:1, :, 0:2], in_=xT_sb[0:1, :, 0, 0:2]
            )
            nc.gpsimd.dma_start(
                out=xres_sb[0:1, :, 0:2], in_=xT_sb[0:1, 0:4, 0, 0:2]
            )
            wo_r = wo_d.ap().rearrange("(t p) n -> p t n", p=128)
            xres_r = xres_d.ap().rearrange("(t p) d -> p t d", p=128)
            for c in range(4):
                nc.gpsimd.dma_start(
                    out=wo_sb[:, 2 * c : 2 * c + 2, :],
                    in_=wo_r[:, 2 * c : 2 * c + 2, :],
                )
            for qt in range(4):
                nc.gpsimd.dma_start(
                    out=xres_sb[:, qt, :], in_=xres_r[:, qt, :]
                )
            if not trivial_gamma:
                nc.gpsimd.dma_start(
                    out=gb_sb,
                    in_=bass.AP(tensor=gamma_d, offset=0, ap=[[0, 128], [1, D]]),
                )
            if not trivial_beta:
                nc.gpsimd.dma_start(
                    out=bb_sb,
                    in_=bass.AP(tensor=beta_d, offset=0, ap=[[0, 128], [1, D]]),
                )
            eps_sb = singles.tile([128, 1], F32)
            nc.vector.memset(eps_sb, eps)

            # ---------------- projection chains ----------------
            qt_sb = singles.tile([128, 2, L], BF16)
            kt_sb = singles.tile([128, 2, L], BF16)
            v8_sb = singles.tile([128, 2, LT, 144], FP8)
            # only the denominator-rider columns (64, 129) must be exactly 1.0
            nc.vector.memset(v8_sb[:, :, :, 64:65], 1.0)
            nc.vector.memset(v8_sb[:, :, :, 129:130], 1.0)

            # projection chains accumulate in tiles from the SAME PSUM ring
            # as the score quads ("sc", bufs=3): no dedicated pj bank, no
            # chain-to-chain serialization through a single accumulator.
            def chain_qk(w_sb, o_sb, b, lc2):
                # one 512-col accumulation group; drain (bf16, x1/64) on
                # VECTOR: the scalar engine's exp stream starves during the
                # chain-heavy early pairs, while vector has more idle there
                cq = psum.tile([128, 1024], F32, tag="sc", bufs=3,
                               name=f"cq_{b}_{lc2}")
                sl = slice(2 * lc2 * 256, (2 * lc2 + 2) * 256)
                for i in range(KD // 2):
                    nc.tensor.matmul(
                        cq[:, 0:512],
                        lhsT=w_sb[:, 2 * i : 2 * i + 2, :],
                        rhs=xT_sb[:, 2 * i : 2 * i + 2, b, sl],
                        start=(i == 0),
                        stop=(i == KD // 2 - 1),
                        perf_mode=DR,
                    )
                nc.vector.tensor_scalar_mul(
                    out=o_sb[:, b, 2 * lc2 * 256 : (2 * lc2 + 2) * 256],
                    in0=cq[:, 0:512],
                    scalar1=1.0 / 64,
                )

            def chain_v(b, lt2):
                # lt tiles 2*lt2, 2*lt2+1 -> cols 0:128 / 512:640 (separate
                # banks); drain on VECTOR (x1/256, fp8 cast) to keep the
                # scalar engine free for exps
                cv = psum.tile([128, 1024], F32, tag="sc", bufs=3,
                               name=f"cv_{b}_{lt2}")
                for half in range(2):
                    lt = 2 * lt2 + half
                    for i in range(KD // 2):
                        nc.tensor.matmul(
                            cv[:, half * 512 : half * 512 + 128],
                            lhsT=xT_sb[:, 2 * i : 2 * i + 2, b,
                                       lt * 128 : (lt + 1) * 128],
                            rhs=wv_sb[:, 2 * i : 2 * i + 2, :],
                            start=(i == 0),
                            stop=(i == KD // 2 - 1),
                            perf_mode=DR,
                        )
                nc.vector.tensor_scalar_mul(
                    out=v8_sb[:, b, 2 * lt2 : 2 * lt2 + 2, 0:130].rearrange(
                        "p t (h a) -> p t h a", h=2
                    )[:, :, :, 0:64],
                    in0=cv.rearrange("p (t x) -> p t x", t=2)[
                        :, :, 0:128
                    ].rearrange("p t (h a) -> p t h a", h=2),
                    scalar1=1.0 / 256,
                )

            # ---------------- attention units ----------------
            cc_in = [dram.tile([8, 130, QW], BF16, name=f"cci{h}") for h in range(2)]
            cc_out = [dram.tile([8 * 130, QW], BF16, name=f"cco{h}") for h in range(2)]
            # per-peer-pair og tiles: Tile deps are tile-granular, so the
            # p-th normalize only waits for its OWN chunk's DMA, not all 4
            og_sb = [
                [singles.tile([128, 2, QW], BF16, name=f"og{h}_{p}")
                 for p in range(4)]
                for h in range(2)
            ]
            S_sb = [singles.tile([128, 8, QW], BF16, name=f"S{h}") for h in range(2)]
            og_n = [
                [singles.tile([128, 2, QW], FP8, name=f"ogn{h}_{p}")
                 for p in range(4)]
                for h in range(2)
            ]

            def attn_pair(b, hi, ha, fillers=()):
                # One PAIR covers units j = hi*4 + 2u + ha (u in {0,1}):
                # two SAME-HALF 256-col q blocks, 512 apart, read as one
                # strided N=512 rhs. Scores and AV run at N=512 (half the
                # PE instruction count of 256-wide units) while each A2A
                # half still completes after its own 4 pairs, keeping the
                # early first collective. po h0/h1 in separate PSUM banks.
                po0 = psum.tile([65, 512], F32, tag="po0", bufs=1,
                                name=f"po0_{b}_{hi}_{ha}")
                po1 = psum.tile([65, 512], F32, tag="po1", bufs=1,
                                name=f"po1_{b}_{hi}_{ha}")
                # layout [65, unit, h, 256]: per-unit slices are contiguous
                # [2h x 256] blocks for the fold/staging DMAs
                po_s = small.tile([65, 2, 2, 256], F32, tag="pos",
                                  name=f"pos_{b}_{hi}_{ha}")
                ex2s = []

                def av_tp(tp):
                    for h, po_h in ((0, po0), (1, po1)):
                        nc.tensor.matmul(
                            po_h,
                            lhsT=v8_sb[:, b, 2 * tp : 2 * tp + 2,
                                       65 * h : 65 * h + 65],
                            rhs=ex2s[tp][:, :, 512 * h : 512 * h + 512],
                            start=(tp == 0),
                            stop=(tp == LT // 2 - 1),
                            perf_mode=DR,
                        )

                for t in range(LT):
                    sc = psum.tile([128, 1024], F32, tag="sc", bufs=3,
                                   name=f"sc_{b}_{hi}_{ha}_{t}")
                    # layout: h0 cols 0:512, h1 cols 512:1024; within each
                    # h: [unit u=0 256 | unit u=1 256]
                    for h in range(2):
                        nc.tensor.matmul(
                            sc[:, 512 * h : 512 * h + 512],
                            lhsT=kt_sb[64 * h : 64 * h + 64, b,
                                       t * 128 : (t + 1) * 128],
                            rhs=qt_sb[64 * h : 64 * h + 64, b, :].rearrange(
                                "p (i u a c) -> p i u a c", i=2, u=2, a=2
                            )[:, hi, :, ha, :],
                            start=True,
                            stop=True,
                        )
                    # AV (one kt-PAIR per DR matmul) trails the exp stream:
                    # its inputs finished long ago, no PE queue stall
                    if t >= 4 and t % 2 == 0:
                        av_tp(t // 2 - 2)
                    if t % 2 == 0:
                        ex2s.append(
                            expp.tile([128, 2, 1024], FP8, tag="ex",
                                      name=f"ex_{b}_{hi}_{ha}_{t // 2}")
                        )
                    ex = ex2s[-1][:, t % 2, :]
                    if t % 2 == 0 or t == LT - 1:
                        nc.scalar.activation(
                            out=ex, in_=sc, func=AF.Exp, scale=0.125
                        )
                    else:
                        nc.vector._custom_dve(
                            EXP_OP, out=ex, in0=sc, s0=A1, s1=A2, imm2=A3
                        )
                    if t < len(fillers):
                        for fn, args in fillers[t]:
                            fn(*args)
                av_tp(LT // 2 - 2)
                av_tp(LT // 2 - 1)
                nc.vector.tensor_copy(out=po_s[:, :, 0, :], in_=po0)
                nc.scalar.activation(
                    out=po_s[:, :, 1, :], in_=po1, func=AF.Copy
                )
                # per 256-col unit u (j = hi*4 + 2u + ha, peer b*4+2*hi+u,
                # A2A half ha): compact reciprocal denominators and stage
                # raw o + 256/den rows into the peer's cc_in shard.
                # Everything on gpsimd SW-DGE (HWDGE concurrent with the
                # first collective's transport stalls ~10us). The tiny
                # folds go FIRST so the vector reciprocals overlap the
                # 128KB og transfers instead of queueing behind them --
                # the rc rows gate the collective trigger.
                rc4s = []
                for u in range(2):
                    rc4 = small.tile([128, 4], F32, tag="rc4",
                                     name=f"rc4_{b}_{hi}_{ha}_{u}")
                    nc.gpsimd.dma_start(
                        out=rc4,
                        in_=po_s[64:65, u, :, :],
                    )
                    rc4s.append(rc4)
                for u in range(2):
                    peer = b * 4 + 2 * hi + u
                    nc.gpsimd.dma_start(
                        out=bass.AP(
                            tensor=cc_in[ha].tensor,
                            offset=peer * 130 * QW,
                            ap=[[QW, 64], [64 * QW, 2], [1, QW]],
                        ),
                        in_=po_s[0:64, u, :, :],
                    )
                for u in range(2):
                    nc.vector.reciprocal(out=rc4s[u], in_=rc4s[u])
                    nc.vector.tensor_scalar_mul(
                        out=rc4s[u], in0=rc4s[u], scalar1=256.0
                    )
                for u in range(2):
                    peer = b * 4 + 2 * hi + u
                    nc.gpsimd.dma_start(
                        out=cc_in[ha][peer, 128:130, :], in_=rc4s[u]
                    )

            # Chains MUST be emitted before any unit that reads their output
            # (Tile deps are emission-ordered). Pace them via per-quad filler
            # slots so the exp stream starts after only 3 chains.
            K_ = lambda b, i: (chain_qk, (wk_sb, kt_sb, b, i))
            Q_ = lambda b, i: (chain_qk, (wq_sb, qt_sb, b, i))
            V_ = lambda b, i: (chain_v, (b, i))

            # even-half pairs first (they fill cc_in[0]); batch-0 chains
            # pace inside the first pairs, batch-1 chains inside the next
            pairs_even = [(0, 0), (0, 1), (1, 0), (1, 1)]
            pairs_odd = [(0, 0), (0, 1), (1, 0), (1, 1)]
            fillmap = {
                (0, 0, 0): [[V_(0, 0), V_(0, 1)], [K_(0, 1)], [V_(0, 2)], [V_(0, 3)],
                            [K_(0, 2)], [V_(0, 4)], [V_(0, 5)], [K_(0, 3)],
                            [V_(0, 6)], [V_(0, 7)], [Q_(0, 2)], [],
                            [Q_(0, 3)]],
                (0, 1, 0): [[K_(1, 0)], [], [Q_(1, 0)], [], [Q_(1, 1)], [],
                            [V_(1, 0)], [], [V_(1, 1)], [], [K_(1, 1)], [],
                            [V_(1, 2)], [], [V_(1, 3)]],
                (1, 0, 0): [[K_(1, 2)], [], [V_(1, 4)], [], [V_(1, 5)], [],
                            [K_(1, 3)], [], [V_(1, 6)], [], [V_(1, 7)], [],
                            [Q_(1, 2)], [], [Q_(1, 3)]],
            }
            def recv_og(half, og_engs, s_eng):
                # S rows FIRST (every normalize needs them), the two h
                # broadcasts split across the queues so both land together
                # S[p, peer, q] = (256/den)[h=p//64, q] of peer's shard
                for h2 in range(2):
                    eng = s_eng if h2 == 0 else og_engs[0]
                    eng.dma_start(
                        out=S_sb[half][64 * h2 : 64 * h2 + 64, :, :],
                        in_=bass.AP(
                            tensor=cc_out[half].tensor,
                            offset=(128 + h2) * QW,
                            ap=[[0, 64], [130 * QW, 8], [1, QW]],
                        ),
                    )
                # one 64KB per-peer DMA each, the tile's two peers split
                # across the queues: post-collective reads are latency-
                # bound, so two parallel half-size transfers complete a
                # tile in roughly half the time of one 2-peer read
                for pr in range(4):
                    for u in range(2):
                        og_engs[(pr + u) % len(og_engs)].dma_start(
                            out=og_sb[half][pr][:, u : u + 1, :],
                            in_=bass.AP(
                                tensor=cc_out[half].tensor,
                                offset=(2 * pr + u) * 130 * QW,
                                ap=[[QW, 128], [1, QW]],
                            ),
                        )

            def og_mult(half):
                # normalize per peer-pair so outproj MM p can start as soon
                # as its own pair is done
                for p in range(4):
                    nc.vector.tensor_mul(
                        out=og_n[half][p],
                        in0=og_sb[half][p],
                        in1=S_sb[half][:, 2 * p : 2 * p + 2, :],
                    )

            # V00 is only needed by AV(0) at kt-slot 4, so it rides in pair
            # 0's first filler slot instead of delaying the first scores
            for fn, args in (K_(0, 0), Q_(0, 0), Q_(0, 1)):
                fn(*args)
            for b, hi in pairs_even:
                attn_pair(b, hi, 0, fillmap.get((b, hi, 0), ()))
            nc.gpsimd.collective_compute(
                "AllToAll",
                mybir.AluOpType.bypass,
                replica_groups=[[0, 1, 2, 3, 4, 5, 6, 7]],
                ins=[cc_in[0].opt()],
                outs=[cc_out[0].opt()],
            )
            for b, hi in pairs_odd:
                attn_pair(b, hi, 1, fillmap.get((b, hi, 1), ()))
            # preload the Sqrt table set while scalar is otherwise idle —
            # the LN sqrts then run without a table load on the tail's
            # critical path (Square is a filler in every set, unaffected).
            # The wait hint pins it between the pairs and the tail blocks:
            # unhinted, the scheduler drifts it into the tail's chain.
            with tc.tile_wait_until(0.99):
                sqpre = small.tile([128, 1], F32, tag="sqpre")
                nc.scalar.activation(
                    out=sqpre, in_=eps_sb, func=AF.Sqrt, bias=0.0, scale=1.0
                )
            # recv0 goes AFTER all stagings (wait hint pins the scheduler's
            # queue order): its A2A#0 wait cleared long ago, so the issues
            # fire immediately and nothing on the gpsimd queue ever blocks
            # behind a collective semaphore
            with tc.tile_wait_until(1.0):
                recv_og(0, [nc.gpsimd], nc.gpsimd)
            nc.gpsimd.collective_compute(
                "AllToAll",
                mybir.AluOpType.bypass,
                replica_groups=[[0, 1, 2, 3, 4, 5, 6, 7]],
                ins=[cc_in[1].opt()],
                outs=[cc_out[1].opt()],
            )
            with tc.tile_wait_until(1.01):
                # og chunk 0 leads on the idle SCALAR queue while the S
                # rows issue on sync in parallel: the first normalize waits
                # on both, so neither queues behind the other
                recv_og(1, [nc.scalar, nc.sync], nc.sync)

            # ---------------- out-projection + residual + LN ----------------
            BN_STATS_DIM = nc.vector.BN_STATS_DIM
            BN_AGGR_DIM = nc.vector.BN_AGGR_DIM
            out_acc = [
                small.tile([128, D], F32, tag="oac", bufs=4, name=f"oac{qt}")
                for qt in range(4)
            ]

            def outproj_qt(half, qt):
                # qt in 0..3 global; local row tile within half: qt%2
                for dmt in range(2):
                    ps = psum.tile([128, 1024], F32, tag="sc", bufs=3,
                                   name=f"op_{qt}_{dmt}")
                    for p in range(4):
                        nc.tensor.matmul(
                            ps[:, 0:512],
                            lhsT=og_n[half][p][
                                :, :, (qt % 2) * 128 : (qt % 2) * 128 + 128
                            ],
                            rhs=wo_sb[:, 2 * p : 2 * p + 2,
                                      dmt * 512 : (dmt + 1) * 512],
                            start=(p == 0),
                            stop=(p == 3),
                            perf_mode=DR,
                        )
                    dsl = slice(dmt * 512, (dmt + 1) * 512)
                    # accum_out: row sums for the LN mean come free
                    nc.vector.scalar_tensor_tensor(
                        out=out_acc[qt][:, dsl],
                        in0=ps[:, 0:512],
                        scalar=1.0 / 4096,
                        in1=xres_sb[:, qt, dsl],
                        op0=MUL,
                        op1=mybir.AluOpType.add,
                        accum_out=s1s[qt][:, dmt : dmt + 1],
                    )

            s1s = [
                small.tile([128, 2], F32, tag="s1", bufs=4, name=f"s1_{qt}")
                for qt in range(4)
            ]
            negmu = [
                small.tile([128, 1], F32, tag="nmu", bufs=4, name=f"nmu{qt}")
                for qt in range(4)
            ]
            sqd = singles.tile([128, 1024], F32, name="sqd")

            def negmu_prep(qt):
                # mean on vector (tiny): negmu = -(s1a + s1b)/1024
                nc.vector.tensor_tensor(
                    out=negmu[qt], in0=s1s[qt][:, 0:1], in1=s1s[qt][:, 1:2],
                    op=mybir.AluOpType.add,
                )
                nc.vector.tensor_scalar_mul(
                    out=negmu[qt], in0=negmu[qt], scalar1=-1.0 / 1024
                )

            def ln_stats_qt(vs2, k, qt):
                # variance on the idle SCALAR engine: Square((x - mu)) with
                # per-partition bias and accum_out. Square is a cheap
                # filler present in every ACT table set: no table load.
                nc.scalar.activation(
                    out=sqd, in_=out_acc[qt], func=AF.Square,
                    bias=negmu[qt], scale=1.0,
                    accum_out=vs2[:, k : k + 1],
                )

            def ln_half(half):
                # per-half finish, fully per-qt pipelined: each row tile's
                # Square -> sqrt -> reciprocal -> apply -> store chain runs
                # as soon as ITS out_acc is ready, instead of the second qt
                # gating the first through a shared [128,2] sqrt. The
                # 1/1024 variance normalization folds into the Sqrt scale;
                # the table was preloaded after the last exp.
                qts = (0, 1) if half == 0 else (2, 3)
                vs2 = small.tile([128, 2], F32, tag=f"vs{half}")
                std2 = small.tile([128, 2], F32, tag=f"std{half}")
                out_r = out_d.ap().rearrange("(t p) d -> p t d", p=128)
                for qt in qts:
                    negmu_prep(qt)
                for k, qt in enumerate(qts):
                    ln_stats_qt(vs2, k, qt)
                    nc.scalar.activation(
                        out=std2[:, k : k + 1], in_=vs2[:, k : k + 1],
                        func=AF.Sqrt, bias=eps_sb, scale=1.0 / 1024,
                    )
                    rstd = std2[:, k : k + 1]
                    nc.vector.reciprocal(out=rstd, in_=rstd)
                    o = out_acc[qt]
                    nc.vector.tensor_scalar(
                        out=o, in0=o,
                        scalar1=negmu[qt], scalar2=rstd,
                        op0=mybir.AluOpType.add, op1=MUL,
                    )
                    if not trivial_gamma:
                        nc.vector.tensor_mul(out=o, in0=o, in1=gb_sb)
                    if not trivial_beta:
                        nc.vector.tensor_add(out=o, in0=o, in1=bb_sb)
                # stores are emitted AFTER both qt chains so the second
                # qt's Square never queues behind the first store's wait on
                # the scalar engine; each store splits across both free
                # queues so the last row tile's data is on the wire in
                # half the time
                for k, qt in enumerate(qts):
                    o = out_acc[qt]
                    e0, e1 = (nc.sync, nc.scalar) if k == 0 else \
                             (nc.scalar, nc.sync)
                    e0.dma_start(out=out_r[:, qt, 0:512], in_=o[:, 0:512])
                    e1.dma_start(out=out_r[:, qt, 512:1024], in_=o[:, 512:1024])

            with tc.tile_wait_until(1.02):
                og_mult(0)
                for qt in (0, 1):
                    outproj_qt(0, qt)
                ln_half(0)
            with tc.tile_wait_until(1.03):
                og_mult(1)
                for qt in (2, 3):
                    outproj_qt(1, qt)
                ln_half(1)
    nc.compile()
    return nc


def make_in_maps(x, Wq, Wk, Wv, Wo, ln_gamma, ln_beta, L, D):
    B = x.shape[0]
    QS = 512
    xT8 = np.ascontiguousarray(
        x.transpose(2, 0, 1).reshape(D, B * L)
    )
    xT8 = np.clip(xT8, -240, 240).astype(E4)
    wo8 = np.clip(Wo * 64.0, -240, 240).astype(E4)
    in_maps = []
    for c in range(8):
        cols = slice(c * 128, (c + 1) * 128)
        bc, qs = c // 4, c % 4
        in_maps.append(
            {
                "xT": xT8,
                "wq": np.clip(
                    np.ascontiguousarray(Wq[:, cols]) * 64.0, -240, 240
                ).astype(E4),
                "wk": np.clip(
                    np.ascontiguousarray(Wk[:, cols]) * 64.0, -240, 240
                ).astype(E4),
                "wv": np.clip(
                    np.ascontiguousarray(Wv[:, cols]) * 64.0, -240, 240
                ).astype(E4),
                "wo": wo8,
                "xres": np.ascontiguousarray(
                    x[bc, qs * QS : (qs + 1) * QS]
                ).astype(np.float32),
                "gamma": np.ascontiguousarray(ln_gamma[None, :]).astype(np.float32),
                "beta": np.ascontiguousarray(ln_beta[None, :]).astype(np.float32),
            }
        )
    return in_maps


def assemble(results, L, D):
    QS = 512
    out = np.zeros((2, L, D), np.float32)
    for c in range(8):
        bc, qs = c // 4, c % 4
        out[bc, qs * QS : (qs + 1) * QS] = results[c]["out"]
    return out


def run(x, Wq, Wk, Wv, Wo, ln_gamma, ln_beta, trace=False):
    B, L, D = x.shape
    nc = build_nc(
        L=L, D=D,
        trivial_gamma=bool(np.all(ln_gamma == 1.0)),
        trivial_beta=bool(np.all(ln_beta == 0.0)),
    )
    in_maps = make_in_maps(x, Wq, Wk, Wv, Wo, ln_gamma, ln_beta, L, D)
    res = bass_utils.run_bass_kernel_spmd(
        nc, in_maps, core_ids=list(range(8)), trace=trace
    )
    return assemble(res.results, L, D), res


def kernel(x, Wq, Wk, Wv, Wo, ln_gamma, ln_beta):
    out, _ = run(
        np.asarray(x, np.float32),
        np.asarray(Wq, np.float32),
        np.asarray(Wk, np.float32),
        np.asarray(Wv, np.float32),
        np.asarray(Wo, np.float32),
        np.asarray(ln_gamma, np.float32),
        np.asarray(ln_beta, np.float32),
    )
    return out



# revision 79
# speedup vs baseline: 1.0084x; 1.0084x over previous
"""MultiHeadSelfAttention Trainium2 kernel (8-core SPMD), v4.

v4 over v2 (the staged baseline):
  - Attention runs as 8 PAIRS of same-A2A-half 256-col units (j and
    j+2, one strided N=512 rhs): score and AV matmuls at N=512 halve
    the PE instruction count (1183 -> ~600 matmuls) while each A2A
    half still completes after its own 4 pairs, preserving the early
    first collective. po h0/h1 live in separate PSUM banks so both AV
    accumulation groups run inline (no h1 burst); AV trails the exp
    stream by 2 kt so the in-order PE queue never stalls on a fresh
    exp. Projection chains allocate from the same PSUM ring as the
    score tiles (no dedicated pj bank, no chain serialization); QK
    chains are one 512-wide DR group, V chains use DoubleRow.
  - Loads: x as 8 half-blocks alternating sync/scalar queues (b0
    first at full HBM bandwidth); wo/xres chunked on the gpsimd queue
    (idle at startup; a big mid-kernel load once stalled rc4 folds
    ~15us behind one multi-MB descriptor).
  - Staging: merged per-unit og DMA, all staging + rc4 folds on
    gpsimd SW-DGE (HWDGE concurrent with a collective transport
    stalls ~10us), phased og-before-folds to minimize cross-engine
    ping-pong.
  - Receive: per-peer-pair og tiles (tile-granular deps let each
    normalize start on its own chunk), S rows first, recv0 pinned
    after all stagings via a wait hint (the scheduler otherwise
    reorders it in front, blocking stagings behind the collective
    semaphore), recv1 issues split across sync+scalar.
  - Tail: LN mean from stt accum_out, variance via scalar Square with
    per-partition -mu bias + accum_out (Square is a filler in every
    ACT table set), Sqrt table preloaded right after the last exp,
    per-half LN finish with stores spread over sync/scalar.
Sharding: 16 heads across 8 cores (2 heads each, "h" in {0,1}); every core
computes BOTH batches for its 2 heads. Output rows: core c owns batch c//4,
q-rows (c%4)*512..+512.

Per-core program (fp8 matmuls, f32 accum): projections from xT fp8
[D, 2b, L] with fp8 weights scaled x64 (Q,K DoubleRow chains -> qt/kt
bf16 x1/64; V DR chains -> v8 fp8 [128 kpos, 2b, 16 lt, 2h x 65+pad],
col 64 of each head block = 1.0 softmax-denominator rider, v scaled
1/256). Attention per pair: 16 kt iterations of 2 score MMs (bf16,
K=64, h-row-tiled concurrent, N=512) -> sc PSUM [128,1024]; exp to
fp8: 56/128 tiles on a custom DVE polynomial (p(s)^4 ~ lam*e^{s/8};
lam mixing across engines stays within tolerance), the rest ScalarE
ACT Exp; AV DoubleRow accumulates po0/po1 [65, 512]. Per unit: drain
po -> po_s f32, fold denominators -> [128,4], reciprocal x256, stage
raw o + 256/den rows (gpsimd f32->bf16 cast DMAs) into cc_in. Two
8-rank AllToAlls ship [8, 130, 256] bf16 shards (fp8 collectives
measured erratic). Receiver: og_mult normalizes per peer pair
(o * 256/den, fp8), out-projection DR with wo fp8 (x64), residual
out = psum/4096 + xres, LayerNorm, store.
"""

import sys

sys.path.insert(0, "/opt/trn_rl_repo")

import numpy as np
import ml_dtypes

import concourse.bass as bass
import concourse.bacc as bacc
import concourse.tile as tile
from concourse import mybir
from concourse import bass_utils
import bass_rust

BF16 = mybir.dt.bfloat16
F32 = mybir.dt.float32
FP8 = mybir.dt.float8e4
AF = mybir.ActivationFunctionType
DR = mybir.MatmulPerfMode.DoubleRow
MUL = mybir.AluOpType.mult
E4 = ml_dtypes.float8_e4m3

# custom DVE exp: p(s) = ((A3 s + A2) s + A1) s + 1;  p^4 ~ LAM * e^{s/8}
A1 = 0.031379391305728067
A2 = 0.00050919663280734283
A3 = 4.9613446254565463e-06

_PATCHED = False


def _patch_tile_drain():
    """The installed walrus rejects >1 sem wait on a Drain instruction; split
    the TileContext tail-drain waits across multiple drains."""
    global _PATCHED
    if _PATCHED:
        return
    _PATCHED = True

    def _patched(self, tick_clock, wait_clock):
        from concourse.vector_clock import ScopedClock

        probe = self.nc.sync.drain()
        wait_clock.add_sem_waits(
            probe.ins, ScopedClock({None: tick_clock.global_clock})
        )
        si = probe.ins.sync_info
        waits = list(si.on_wait or []) if si is not None else []
        if len(waits) > 1:
            si.on_wait = [waits[0]]
            for w in waits[1:]:
                d2 = self.nc.sync.drain()
                si2 = d2.ins.sync_info
                if si2 is None:
                    d2.ins.sync_info = bass_rust.SyncInfo(on_wait=[w], on_update=[])
                else:
                    si2.on_wait = [w]
        self.nc.all_engine_barrier()
        assert self.sems is not None
        popped = self.nc._tile_sem_poison_stack.pop()
        assert popped is self._sem_poison
        self.nc.clear_and_free_semaphores(list(self.sems.allocated().values()))
        self.nc.all_engine_barrier()

    tile.TileContext._drain_and_barrier = _patched


def _register_exp_op():
    """Register the polynomial-exp custom DVE op (append-only; idempotent)."""
    from concourse import dve_ops
    from concourse.dve_spec import Spec, Src0, C0, C1, C2, One, sq, lower
    from concourse.dve_uop import DveOpSpec
    from concourse.dve_ops import DveOp

    name = "EXP_S8_ANT"
    if name in dve_ops._SUB_OPCODE_FOR_NAME:
        return next(o for o in dve_ops.OPS if o.name == name)

    x = Src0
    body = sq(sq(((C2 * x + C1) * x + C0) * x + One))

    def ref(in0, in1, s0, s1, imm2):
        return ((((imm2 * in0 + s1) * in0 + s0) * in0 + 1.0) ** 2) ** 2

    spec = Spec(body=body, reference=ref)
    row = dve_ops._CUSTOM_DVE_ROW_BASE + len(dve_ops.OPS)
    shas = {}
    for ver in ("v3", "v4"):
        compiled = DveOpSpec(
            name=name, opcode=row, uops=lower(spec, ver=ver), rd1_en=False
        )
        shas[ver] = compiled.sha(ver)
    op = DveOp(name, spec, subdim=False, uops_sha=shas)
    dve_ops.OPS.append(op)
    dve_ops._SUB_OPCODE_FOR_NAME[name] = row
    dve_ops.CUSTOM_DVE_SPECS[name] = spec
    return op


def build_nc(L=2048, D=1024, eps=1e-6, trivial_gamma=False, trivial_beta=False):
    _patch_tile_drain()
    EXP_OP = _register_exp_op()

    KD = D // 128     # 8 contraction tiles over D
    LT = L // 128     # 16 kpos tiles
    NJ = L // 256     # 8 q-chunks of 256 per batch
    QW = 256          # unit q width
    QS = 512          # per-core output rows

    nc = bacc.Bacc(num_devices=8, debug=False)

    xT_d = nc.dram_tensor("xT", [D, 2 * L], FP8, kind="ExternalInput")
    wq_d = nc.dram_tensor("wq", [D, 128], FP8, kind="ExternalInput")
    wk_d = nc.dram_tensor("wk", [D, 128], FP8, kind="ExternalInput")
    wv_d = nc.dram_tensor("wv", [D, 128], FP8, kind="ExternalInput")
    wo_d = nc.dram_tensor("wo", [D, D], FP8, kind="ExternalInput")
    xres_d = nc.dram_tensor("xres", [QS, D], F32, kind="ExternalInput")
    gamma_d = nc.dram_tensor("gamma", [1, D], F32, kind="ExternalInput")
    beta_d = nc.dram_tensor("beta", [1, D], F32, kind="ExternalInput")
    out_d = nc.dram_tensor("out", [QS, D], F32, kind="ExternalOutput")

    with tile.TileContext(nc) as tc:
        with (
            tc.tile_pool(name="singles", bufs=1) as singles,
            tc.tile_pool(name="exp", bufs=10) as expp,
            tc.tile_pool(name="small", bufs=2) as small,
            tc.tile_pool(name="psum", bufs=1, space="PSUM") as psum,
            tc.tile_pool(name="dram", bufs=1, space="DRAM") as dram,
        ):
            # ---------------- loads ----------------
            xT_sb = singles.tile([128, KD, 2, L], FP8)
            wq_sb = singles.tile([128, KD, 128], FP8)
            wk_sb = singles.tile([128, KD, 128], FP8)
            wv_sb = singles.tile([128, KD, 128], FP8)
            for eng, w_sb, w_d in (
                (nc.sync, wk_sb, wk_d),
                (nc.scalar, wq_sb, wq_d),
            ):
                eng.dma_start(
                    out=w_sb, in_=w_d.ap().rearrange("(t p) m -> p t m", p=128)
                )
            # x loads: each (b, lh) 1MB block split across BOTH queues so the
            # block in flight gets the full HBM bandwidth, strictly ordered
            # b0-lh0 first (what the first chains need). wv slots in after
            # the b0-lh0 half: the V chain runs third, after K and Q.
            xT_r = xT_d.ap().rearrange("(t p) m -> p t m", p=128)
            for b in range(2):
                for lh in range(2):
                    for tq, eng in ((0, nc.sync), (4, nc.scalar)):
                        nc_sl = slice(b * L + lh * 1024, b * L + (lh + 1) * 1024)
                        eng.dma_start(
                            out=xT_sb[:, tq : tq + 4, b,
                                      lh * 1024 : (lh + 1) * 1024],
                            in_=xT_r[:, tq : tq + 4, nc_sl],
                        )
                    if b == 0 and lh == 0:
                        nc.scalar.dma_start(
                            out=wv_sb,
                            in_=wv_d.ap().rearrange("(t p) m -> p t m", p=128),
                        )
            wo_sb = singles.tile([128, KD, D], FP8)
            xres_sb = singles.tile([128, 4, D], F32)
            gb_sb = singles.tile([128, D], F32)
            bb_sb = singles.tile([128, D], F32)
            # wo/xres loads go on the GPSIMD queue: it is idle until unit-0
            # staging (~30us), is not in the startup critical path (sync/
            # scalar queues gate the first chains), and owns no per-unit
            # small DMAs that could get stuck behind a multi-MB descriptor.
            # Chunked so its own staging DMAs can slip in at boundaries.
            # Probe DMAs first: each writes a corner of wo_sb/xres_sb from
            # xT(b0, lh0), so via WAW deps the 3MB wo/xres stream stays off
            # the HBM until the first chains' data has landed (otherwise
            # the PE stalls ~13us at startup). The probed bytes are
            # overwritten by the real loads right after.
            nc.gpsimd.dma_start(
                out=wo_sb[0:1, :, 0:2], in_=xT_sb[0:1, :, 0, 0:2]
            )
            nc.gpsimd.dma_start(
                out=xres_sb[0:1, :, 0:2], in_=xT_sb[0:1, 0:4, 0, 0:2]
            )
            wo_r = wo_d.ap().rearrange("(t p) n -> p t n", p=128)
            xres_r = xres_d.ap().rearrange("(t p) d -> p t d", p=128)
            for c in range(4):
                nc.gpsimd.dma_start(
                    out=wo_sb[:, 2 * c : 2 * c + 2, :],
                    in_=wo_r[:, 2 * c : 2 * c + 2, :],
                )
            for qt in range(4):
                nc.gpsimd.dma_start(
                    out=xres_sb[:, qt, :], in_=xres_r[:, qt, :]
                )
            if not trivial_gamma:
                nc.gpsimd.dma_start(
                    out=gb_sb,
                    in_=bass.AP(tensor=gamma_d, offset=0, ap=[[0, 128], [1, D]]),
                )
            if not trivial_beta:
                nc.gpsimd.dma_start(
                    out=bb_sb,
                    in_=bass.AP(tensor=beta_d, offset=0, ap=[[0, 128], [1, D]]),
                )
            eps_sb = singles.tile([128, 1], F32)
            nc.vector.memset(eps_sb, eps)

            # ---------------- projection chains ----------------
            qt_sb = singles.tile([128, 2, L], BF16)
            kt_sb = singles.tile([128, 2, L], BF16)
            v8_sb = singles.tile([128, 2, LT, 144], FP8)
            # only the denominator-rider columns (64, 129) must be exactly 1.0
            nc.vector.memset(v8_sb[:, :, :, 64:65], 1.0)
            nc.vector.memset(v8_sb[:, :, :, 129:130], 1.0)

            # projection chains accumulate in tiles from the SAME PSUM ring
            # as the score quads ("sc", bufs=3): no dedicated pj bank, no
            # chain-to-chain serialization through a single accumulator.
            def chain_qk(w_sb, o_sb, b, lc2):
                # one 512-col accumulation group; drain (bf16, x1/64) on
                # VECTOR: the scalar engine's exp stream starves during the
                # chain-heavy early pairs, while vector has more idle there
                cq = psum.tile([128, 1024], F32, tag="sc", bufs=3,
                               name=f"cq_{b}_{lc2}")
                sl = slice(2 * lc2 * 256, (2 * lc2 + 2) * 256)
                for i in range(KD // 2):
                    nc.tensor.matmul(
                        cq[:, 0:512],
                        lhsT=w_sb[:, 2 * i : 2 * i + 2, :],
                        rhs=xT_sb[:, 2 * i : 2 * i + 2, b, sl],
                        start=(i == 0),
                        stop=(i == KD // 2 - 1),
                        perf_mode=DR,
                    )
                nc.vector.tensor_scalar_mul(
                    out=o_sb[:, b, 2 * lc2 * 256 : (2 * lc2 + 2) * 256],
                    in0=cq[:, 0:512],
                    scalar1=1.0 / 64,
                )

            def chain_v(b, lt2):
                # lt tiles 2*lt2, 2*lt2+1 -> cols 0:128 / 512:640 (separate
                # banks); drain on VECTOR (x1/256, fp8 cast) to keep the
                # scalar engine free for exps
                cv = psum.tile([128, 1024], F32, tag="sc", bufs=3,
                               name=f"cv_{b}_{lt2}")
                for half in range(2):
                    lt = 2 * lt2 + half
                    for i in range(KD // 2):
                        nc.tensor.matmul(
                            cv[:, half * 512 : half * 512 + 128],
                            lhsT=xT_sb[:, 2 * i : 2 * i + 2, b,
                                       lt * 128 : (lt + 1) * 128],
                            rhs=wv_sb[:, 2 * i : 2 * i + 2, :],
                            start=(i == 0),
                            stop=(i == KD // 2 - 1),
                            perf_mode=DR,
                        )
                nc.vector.tensor_scalar_mul(
                    out=v8_sb[:, b, 2 * lt2 : 2 * lt2 + 2, 0:130].rearrange(
                        "p t (h a) -> p t h a", h=2
                    )[:, :, :, 0:64],
                    in0=cv.rearrange("p (t x) -> p t x", t=2)[
                        :, :, 0:128
                    ].rearrange("p t (h a) -> p t h a", h=2),
                    scalar1=1.0 / 256,
                )

            # ---------------- attention units ----------------
            cc_in = [dram.tile([8, 130, QW], BF16, name=f"cci{h}") for h in range(2)]
            cc_out = [dram.tile([8 * 130, QW], BF16, name=f"cco{h}") for h in range(2)]
            # per-peer-pair og tiles: Tile deps are tile-granular, so the
            # p-th normalize only waits for its OWN chunk's DMA, not all 4
            og_sb = [
                [singles.tile([128, 2, QW], BF16, name=f"og{h}_{p}")
                 for p in range(4)]
                for h in range(2)
            ]
            S_sb = [singles.tile([128, 8, QW], BF16, name=f"S{h}") for h in range(2)]
            og_n = [
                [singles.tile([128, 2, QW], FP8, name=f"ogn{h}_{p}")
                 for p in range(4)]
                for h in range(2)
            ]

            def attn_pair(b, hi, ha, fillers=()):
                # One PAIR covers units j = hi*4 + 2u + ha (u in {0,1}):
                # two SAME-HALF 256-col q blocks, 512 apart, read as one
                # strided N=512 rhs. Scores and AV run at N=512 (half the
                # PE instruction count of 256-wide units) while each A2A
                # half still completes after its own 4 pairs, keeping the
                # early first collective. po h0/h1 in separate PSUM banks.
                po0 = psum.tile([65, 512], F32, tag="po0", bufs=1,
                                name=f"po0_{b}_{hi}_{ha}")
                po1 = psum.tile([65, 512], F32, tag="po1", bufs=1,
                                name=f"po1_{b}_{hi}_{ha}")
                # layout [65, unit, h, 256]: per-unit slices are contiguous
                # [2h x 256] blocks for the fold/staging DMAs
                po_s = small.tile([65, 2, 2, 256], F32, tag="pos",
                                  name=f"pos_{b}_{hi}_{ha}")
                ex2s = []

                def av_tp(tp):
                    for h, po_h in ((0, po0), (1, po1)):
                        nc.tensor.matmul(
                            po_h,
                            lhsT=v8_sb[:, b, 2 * tp : 2 * tp + 2,
                                       65 * h : 65 * h + 65],
                            rhs=ex2s[tp][:, :, 512 * h : 512 * h + 512],
                            start=(tp == 0),
                            stop=(tp == LT // 2 - 1),
                            perf_mode=DR,
                        )

                for t in range(LT):
                    sc = psum.tile([128, 1024], F32, tag="sc", bufs=3,
                                   name=f"sc_{b}_{hi}_{ha}_{t}")
                    # layout: h0 cols 0:512, h1 cols 512:1024; within each
                    # h: [unit u=0 256 | unit u=1 256]
                    for h in range(2):
                        nc.tensor.matmul(
                            sc[:, 512 * h : 512 * h + 512],
                            lhsT=kt_sb[64 * h : 64 * h + 64, b,
                                       t * 128 : (t + 1) * 128],
                            rhs=qt_sb[64 * h : 64 * h + 64, b, :].rearrange(
                                "p (i u a c) -> p i u a c", i=2, u=2, a=2
                            )[:, hi, :, ha, :],
                            start=True,
                            stop=True,
                        )
                    # AV (one kt-PAIR per DR matmul) trails the exp stream:
                    # its inputs finished long ago, no PE queue stall
                    if t >= 4 and t % 2 == 0:
                        av_tp(t // 2 - 2)
                    if t % 2 == 0:
                        ex2s.append(
                            expp.tile([128, 2, 1024], FP8, tag="ex",
                                      name=f"ex_{b}_{hi}_{ha}_{t // 2}")
                        )
                    ex = ex2s[-1][:, t % 2, :]
                    if t % 2 == 0 or t == LT - 1:
                        nc.scalar.activation(
                            out=ex, in_=sc, func=AF.Exp, scale=0.125
                        )
                    else:
                        nc.vector._custom_dve(
                            EXP_OP, out=ex, in0=sc, s0=A1, s1=A2, imm2=A3
                        )
                    if t < len(fillers):
                        for fn, args in fillers[t]:
                            fn(*args)
                av_tp(LT // 2 - 2)
                av_tp(LT // 2 - 1)
                nc.vector.tensor_copy(out=po_s[:, :, 0, :], in_=po0)
                nc.scalar.activation(
                    out=po_s[:, :, 1, :], in_=po1, func=AF.Copy
                )
                # per 256-col unit u (j = hi*4 + 2u + ha, peer b*4+2*hi+u,
                # A2A half ha): compact reciprocal denominators and stage
                # raw o + 256/den rows into the peer's cc_in shard.
                # Everything on gpsimd SW-DGE (HWDGE concurrent with the
                # first collective's transport stalls ~10us). The tiny
                # folds go FIRST so the vector reciprocals overlap the
                # 128KB og transfers instead of queueing behind them --
                # the rc rows gate the collective trigger.
                rc4s = []
                for u in range(2):
                    rc4 = small.tile([128, 4], F32, tag="rc4",
                                     name=f"rc4_{b}_{hi}_{ha}_{u}")
                    nc.gpsimd.dma_start(
                        out=rc4,
                        in_=po_s[64:65, u, :, :],
                    )
                    rc4s.append(rc4)
                for u in range(2):
                    peer = b * 4 + 2 * hi + u
                    nc.gpsimd.dma_start(
                        out=bass.AP(
                            tensor=cc_in[ha].tensor,
                            offset=peer * 130 * QW,
                            ap=[[QW, 64], [64 * QW, 2], [1, QW]],
                        ),
                        in_=po_s[0:64, u, :, :],
                    )
                for u in range(2):
                    nc.vector.reciprocal(out=rc4s[u], in_=rc4s[u])
                    nc.vector.tensor_scalar_mul(
                        out=rc4s[u], in0=rc4s[u], scalar1=256.0
                    )
                for u in range(2):
                    peer = b * 4 + 2 * hi + u
                    nc.gpsimd.dma_start(
                        out=cc_in[ha][peer, 128:130, :], in_=rc4s[u]
                    )

            # Chains MUST be emitted before any unit that reads their output
            # (Tile deps are emission-ordered). Pace them via per-quad filler
            # slots so the exp stream starts after only 3 chains.
            K_ = lambda b, i: (chain_qk, (wk_sb, kt_sb, b, i))
            Q_ = lambda b, i: (chain_qk, (wq_sb, qt_sb, b, i))
            V_ = lambda b, i: (chain_v, (b, i))

            # even-half pairs first (they fill cc_in[0]); batch-0 chains
            # pace inside the first pairs, batch-1 chains inside the next
            pairs_even = [(0, 0), (0, 1), (1, 0), (1, 1)]
            pairs_odd = [(0, 0), (0, 1), (1, 0), (1, 1)]
            fillmap = {
                (0, 0, 0): [[V_(0, 0), V_(0, 1)], [K_(0, 1)], [V_(0, 2)], [V_(0, 3)],
                            [K_(0, 2)], [V_(0, 4)], [V_(0, 5)], [K_(0, 3)],
                            [V_(0, 6)], [V_(0, 7)], [Q_(0, 2)], [],
                            [Q_(0, 3)]],
                (0, 1, 0): [[K_(1, 0)], [], [Q_(1, 0)], [], [Q_(1, 1)], [],
                            [V_(1, 0)], [], [V_(1, 1)], [], [K_(1, 1)], [],
                            [V_(1, 2)], [], [V_(1, 3)]],
                (1, 0, 0): [[K_(1, 2)], [], [V_(1, 4)], [], [V_(1, 5)], [],
                            [K_(1, 3)], [], [V_(1, 6)], [], [V_(1, 7)], [],
                            [Q_(1, 2)], [], [Q_(1, 3)]],
            }
            def recv_og(half, og_engs, s_eng):
                # S rows FIRST (every normalize needs them; they're tiny),
                # then one 2-peer chunk per og tile so later pairs stream
                # while earlier ones already normalize
                # S[p, peer, q] = (256/den)[h=p//64, q] of peer's shard
                for h2 in range(2):
                    s_eng.dma_start(
                        out=S_sb[half][64 * h2 : 64 * h2 + 64, :, :],
                        in_=bass.AP(
                            tensor=cc_out[half].tensor,
                            offset=(128 + h2) * QW,
                            ap=[[0, 64], [130 * QW, 8], [1, QW]],
                        ),
                    )
                # one 64KB per-peer DMA each, the tile's two peers split
                # across the queues: post-collective reads are latency-
                # bound, so two parallel half-size transfers complete a
                # tile in roughly half the time of one 2-peer read
                for pr in range(4):
                    for u in range(2):
                        og_engs[(pr + u) % len(og_engs)].dma_start(
                            out=og_sb[half][pr][:, u : u + 1, :],
                            in_=bass.AP(
                                tensor=cc_out[half].tensor,
                                offset=(2 * pr + u) * 130 * QW,
                                ap=[[QW, 128], [1, QW]],
                            ),
                        )

            def og_mult(half):
                # normalize per peer-pair so outproj MM p can start as soon
                # as its own pair is done
                for p in range(4):
                    nc.vector.tensor_mul(
                        out=og_n[half][p],
                        in0=og_sb[half][p],
                        in1=S_sb[half][:, 2 * p : 2 * p + 2, :],
                    )

            # V00 is only needed by AV(0) at kt-slot 4, so it rides in pair
            # 0's first filler slot instead of delaying the first scores
            for fn, args in (K_(0, 0), Q_(0, 0), Q_(0, 1)):
                fn(*args)
            for b, hi in pairs_even:
                attn_pair(b, hi, 0, fillmap.get((b, hi, 0), ()))
            nc.gpsimd.collective_compute(
                "AllToAll",
                mybir.AluOpType.bypass,
                replica_groups=[[0, 1, 2, 3, 4, 5, 6, 7]],
                ins=[cc_in[0].opt()],
                outs=[cc_out[0].opt()],
            )
            for b, hi in pairs_odd:
                attn_pair(b, hi, 1, fillmap.get((b, hi, 1), ()))
            # preload the Sqrt table set while scalar is otherwise idle —
            # the LN sqrts then run without a table load on the tail's
            # critical path (Square is a filler in every set, unaffected).
            # The wait hint pins it between the pairs and the tail blocks:
            # unhinted, the scheduler drifts it into the tail's chain.
            with tc.tile_wait_until(0.99):
                sqpre = small.tile([128, 1], F32, tag="sqpre")
                nc.scalar.activation(
                    out=sqpre, in_=eps_sb, func=AF.Sqrt, bias=0.0, scale=1.0
                )
            # recv0 goes AFTER all stagings (wait hint pins the scheduler's
            # queue order): its A2A#0 wait cleared long ago, so the issues
            # fire immediately and nothing on the gpsimd queue ever blocks
            # behind a collective semaphore
            with tc.tile_wait_until(1.0):
                recv_og(0, [nc.gpsimd], nc.gpsimd)
            nc.gpsimd.collective_compute(
                "AllToAll",
                mybir.AluOpType.bypass,
                replica_groups=[[0, 1, 2, 3, 4, 5, 6, 7]],
                ins=[cc_in[1].opt()],
                outs=[cc_out[1].opt()],
            )
            with tc.tile_wait_until(1.01):
                # og chunk 0 leads on the idle SCALAR queue while the S
                # rows issue on sync in parallel: the first normalize waits
                # on both, so neither queues behind the other
                recv_og(1, [nc.scalar, nc.sync], nc.sync)

            # ---------------- out-projection + residual + LN ----------------
            BN_STATS_DIM = nc.vector.BN_STATS_DIM
            BN_AGGR_DIM = nc.vector.BN_AGGR_DIM
            out_acc = [
                small.tile([128, D], F32, tag="oac", bufs=4, name=f"oac{qt}")
                for qt in range(4)
            ]

            def outproj_qt(half, qt):
                # qt in 0..3 global; local row tile within half: qt%2
                for dmt in range(2):
                    ps = psum.tile([128, 1024], F32, tag="sc", bufs=3,
                                   name=f"op_{qt}_{dmt}")
                    for p in range(4):
                        nc.tensor.matmul(
                            ps[:, 0:512],
                            lhsT=og_n[half][p][
                                :, :, (qt % 2) * 128 : (qt % 2) * 128 + 128
                            ],
                            rhs=wo_sb[:, 2 * p : 2 * p + 2,
                                      dmt * 512 : (dmt + 1) * 512],
                            start=(p == 0),
                            stop=(p == 3),
                            perf_mode=DR,
                        )
                    dsl = slice(dmt * 512, (dmt + 1) * 512)
                    # accum_out: row sums for the LN mean come free
                    nc.vector.scalar_tensor_tensor(
                        out=out_acc[qt][:, dsl],
                        in0=ps[:, 0:512],
                        scalar=1.0 / 4096,
                        in1=xres_sb[:, qt, dsl],
                        op0=MUL,
                        op1=mybir.AluOpType.add,
                        accum_out=s1s[qt][:, dmt : dmt + 1],
                    )

            s1s = [
                small.tile([128, 2], F32, tag="s1", bufs=4, name=f"s1_{qt}")
                for qt in range(4)
            ]
            negmu = [
                small.tile([128, 1], F32, tag="nmu", bufs=4, name=f"nmu{qt}")
                for qt in range(4)
            ]
            sqd = singles.tile([128, 1024], F32, name="sqd")

            def negmu_prep(qt):
                # mean on vector (tiny): negmu = -(s1a + s1b)/1024
                nc.vector.tensor_tensor(
                    out=negmu[qt], in0=s1s[qt][:, 0:1], in1=s1s[qt][:, 1:2],
                    op=mybir.AluOpType.add,
                )
                nc.vector.tensor_scalar_mul(
                    out=negmu[qt], in0=negmu[qt], scalar1=-1.0 / 1024
                )

            def ln_stats_qt(vs2, k, qt):
                # variance on the idle SCALAR engine: Square((x - mu)) with
                # per-partition bias and accum_out. Square is a cheap
                # filler present in every ACT table set: no table load.
                nc.scalar.activation(
                    out=sqd, in_=out_acc[qt], func=AF.Square,
                    bias=negmu[qt], scale=1.0,
                    accum_out=vs2[:, k : k + 1],
                )

            def ln_half(half):
                # per-half finish, fully per-qt pipelined: each row tile's
                # Square -> sqrt -> reciprocal -> apply -> store chain runs
                # as soon as ITS out_acc is ready, instead of the second qt
                # gating the first through a shared [128,2] sqrt. The
                # 1/1024 variance normalization folds into the Sqrt scale;
                # the table was preloaded after the last exp.
                qts = (0, 1) if half == 0 else (2, 3)
                vs2 = small.tile([128, 2], F32, tag=f"vs{half}")
                std2 = small.tile([128, 2], F32, tag=f"std{half}")
                out_r = out_d.ap().rearrange("(t p) d -> p t d", p=128)
                for qt in qts:
                    negmu_prep(qt)
                for k, qt in enumerate(qts):
                    ln_stats_qt(vs2, k, qt)
                    nc.scalar.activation(
                        out=std2[:, k : k + 1], in_=vs2[:, k : k + 1],
                        func=AF.Sqrt, bias=eps_sb, scale=1.0 / 1024,
                    )
                    rstd = std2[:, k : k + 1]
                    nc.vector.reciprocal(out=rstd, in_=rstd)
                    o = out_acc[qt]
                    nc.vector.tensor_scalar(
                        out=o, in0=o,
                        scalar1=negmu[qt], scalar2=rstd,
                        op0=mybir.AluOpType.add, op1=MUL,
                    )
                    if not trivial_gamma:
                        nc.vector.tensor_mul(out=o, in0=o, in1=gb_sb)
                    if not trivial_beta:
                        nc.vector.tensor_add(out=o, in0=o, in1=bb_sb)
                # stores are emitted AFTER both qt chains so the second
                # qt's Square never queues behind the first store's wait on
                # the scalar engine; each store splits across both free
                # queues so the last row tile's data is on the wire in
                # half the time
                for k, qt in enumerate(qts):
                    o = out_acc[qt]
                    e0, e1 = (nc.sync, nc.scalar) if k == 0 else \
                             (nc.scalar, nc.sync)
                    e0.dma_start(out=out_r[:, qt, 0:512], in_=o[:, 0:512])
                    e1.dma_start(out=out_r[:, qt, 512:1024], in_=o[:, 512:1024])

            with tc.tile_wait_until(1.02):
                og_mult(0)
                for qt in (0, 1):
                    outproj_qt(0, qt)
                ln_half(0)
            with tc.tile_wait_until(1.03):
                og_mult(1)
                for qt in (2, 3):
                    outproj_qt(1, qt)
                ln_half(1)
    nc.compile()
    return nc


def make_in_maps(x, Wq, Wk, Wv, Wo, ln_gamma, ln_beta, L, D):
    B = x.shape[0]
    QS = 512
    xT8 = np.ascontiguousarray(
        x.transpose(2, 0, 1).reshape(D, B * L)
    )
    xT8 = np.clip(xT8, -240, 240).astype(E4)
    wo8 = np.clip(Wo * 64.0, -240, 240).astype(E4)
    in_maps = []
    for c in range(8):
        cols = slice(c * 128, (c + 1) * 128)
        bc, qs = c // 4, c % 4
        in_maps.append(
            {
                "xT": xT8,
                "wq": np.clip(
                    np.ascontiguousarray(Wq[:, cols]) * 64.0, -240, 240
                ).astype(E4),
                "wk": np.clip(
                    np.ascontiguousarray(Wk[:, cols]) * 64.0, -240, 240
                ).astype(E4),
                "wv": np.clip(
                    np.ascontiguousarray(Wv[:, cols]) * 64.0, -240, 240
                ).astype(E4),
                "wo": wo8,
                "xres": np.ascontiguousarray(
                    x[bc, qs * QS : (qs + 1) * QS]
                ).astype(np.float32),
                "gamma": np.ascontiguousarray(ln_gamma[None, :]).astype(np.float32),
                "beta": np.ascontiguousarray(ln_beta[None, :]).astype(np.float32),
            }
        )
    return in_maps


def assemble(results, L, D):
    QS = 512
    out = np.zeros((2, L, D), np.float32)
    for c in range(8):
        bc, qs = c // 4, c % 4
        out[bc, qs * QS : (qs + 1) * QS] = results[c]["out"]
    return out


def run(x, Wq, Wk, Wv, Wo, ln_gamma, ln_beta, trace=False):
    B, L, D = x.shape
    nc = build_nc(
        L=L, D=D,
        trivial_gamma=bool(np.all(ln_gamma == 1.0)),
        trivial_beta=bool(np.all(ln_beta == 0.0)),
    )
    in_maps = make_in_maps(x, Wq, Wk, Wv, Wo, ln_gamma, ln_beta, L, D)
    res = bass_utils.run_bass_kernel_spmd(
        nc, in_maps, core_ids=list(range(8)), trace=trace
    )
    return assemble(res.results, L, D), res


def kernel(x, Wq, Wk, Wv, Wo, ln_gamma, ln_beta):
    out, _ = run(
        np.asarray(x, np.float32),
        np.asarray(Wq, np.float32),
        np.asarray(Wk, np.float32),
        np.asarray(Wv, np.float32),
        np.asarray(Wo, np.float32),
        np.asarray(ln_gamma, np.float32),
        np.asarray(ln_beta, np.float32),
    )
    return out

